# revision 28
# baseline (speedup 1.0000x reference)
"""Self-contained Trainium2 Bass kernel for nn_CustomAttention_35278861369702.

Computation (see problem reference): causal GQA attention with RoPE.
  B=2, S=2048, H=2048, NH=16 q-heads, NKV=4 kv-heads, HD=128.

Sharding: 8 cores = 2 batches x 4 GQA groups. Core c handles batch c//4 and
q-heads 4g..4g+3 / kv-head g where g = c%4. Wq/Wk/Wv column-parallel,
Wo row-parallel; per-core partial outputs are summed on the host.

Device-side layout: everything transposed (no PE transposes of the softmax
matrix needed).
  - Projections produce Q^T/K^T/V^T [hd, s] (x^T as moving operand);
    chunk passes: (K,V) then (Q0,Q1) then (Q2,Q3) so attention data is
    ready earliest and startup DMA needs only wk/wv.
  - Scores computed as S^T [k, q] into PAIRED 2-bank PSUM tiles; ONE exp
    per pair on ACT ([128,1024] amortizes the 352-cycle ACT overhead);
    causal handled by trimming the moving q-range of diagonal k-tiles
    plus [128,128] triangular bf16 mask multiplies on DVE.
  - Softmax sums: diag tiles via cheap PE ones-matmuls; full-tile pairs
    accumulated in bf16 SBUF on GPSIMD (DVE stays free for evacuations);
    reciprocal on DVE (vector.reciprocal), broadcast via one PE matmul.
  - AV: lhsT=V tile [k, d], rhs=P^T [k, q] -> out^T [d, q], software
    pipelined so PE does not wait on ACT's exp.
  - O-projection paired (2 output tiles per 2-bank PSUM slot, one wide
    DVE cast), woven into the NEXT attention chunk; ascending chunk
    order (0..3) so every slot is PE-bound; oproj(3) is the tail.
  - All DRAM I/O uses host-prepacked contiguous blocks (full-rate DMA).
"""

import math
import sys
import types

sys.path.insert(0, "/opt/trn_rl_repo")

import numpy as np

import concourse.bass as bass
import concourse.mybir as mybir
import concourse.tile as tile
from concourse.bass_utils import run_bass_kernel_spmd

B, S, H = 2, 2048, 2048
NH, NKV, HD = 16, 4, 128
THETA = 10000.0
NCORES = 8
GROUPS = 4          # kv groups (= cores per batch)
HPG = NH // NKV     # q heads per group = 4
DQ = HPG * HD       # per-core q projection width = 512
SC = 512            # s-chunk (moving dim) for projections / attention
NSC = S // SC       # 4
NHT = H // 128      # 16 h-tiles (contraction tiles)
NST = S // 128      # 16 s-tiles / k-tiles
SCALE = 1.0 / math.sqrt(HD)

F32 = mybir.dt.float32
BF16 = mybir.dt.bfloat16


def _legalize_waits(nc):
    """This container's walrus accepts at most ONE sync wait per instruction.
    Split extra waits onto InstEventSemaphore carriers in engine order."""
    n = 0
    for f in nc.m.functions:
        for bb in f.blocks:
            new_insts = []
            for inst in bb.instructions:
                si = inst.sync_info
                if si and si.on_wait and len(si.on_wait) > 1:
                    waits = list(si.on_wait)
                    for j, w in enumerate(waits[:-1]):
                        es = mybir.InstEventSemaphore(
                            name=f"{inst.name}-wsplit{j}",
                            engine=inst.engine,
                            ins=[],
                            outs=[],
                            sync_info=mybir.SyncInfo(on_wait=[w], on_update=[]),
                        )
                        nc.register_instruction(es)
                        new_insts.append(es)
                        n += 1
                    si.on_wait = [waits[-1]]
                new_insts.append(inst)
            bb.instructions[:] = new_insts
    return n


def build_nc():
    nc = bass.Bass()

    # ---- DRAM I/O (per-core shards; same program on all 8 cores) ----
    # All blocks are host-prepacked to the exact SBUF layout so every DMA
    # is contiguous.
    xb = nc.dram_tensor("xb", [NSC, NHT, 128, SC], BF16, kind="ExternalInput")
    wq = nc.dram_tensor("wq", [128, NHT, DQ], BF16, kind="ExternalInput")
    wk = nc.dram_tensor("wk", [128, NHT, HD], BF16, kind="ExternalInput")
    wv = nc.dram_tensor("wv", [128, NHT, HD], BF16, kind="ExternalInput")
    wo = nc.dram_tensor("wo", [128, HPG, H], BF16, kind="ExternalInput")
    cosT = nc.dram_tensor("cosT", [128, S], BF16, kind="ExternalInput")
    sinT = nc.dram_tensor("sinT", [128, S], BF16, kind="ExternalInput")
    pmat = nc.dram_tensor("pmat", [HD, HD], BF16, kind="ExternalInput")
    ident = nc.dram_tensor("ident", [128, 128], BF16, kind="ExternalInput")
    trimask = nc.dram_tensor("trimask", [128, 128], BF16, kind="ExternalInput")
    ones = nc.dram_tensor("ones", [128, 1], BF16, kind="ExternalInput")
    onesrow = nc.dram_tensor("onesrow", [1, 128], BF16, kind="ExternalInput")

    yb = nc.dram_tensor("yb", [NSC, NST, 128, SC], BF16, kind="ExternalOutput")

    from contextlib import ExitStack

    with tile.TileContext(nc) as tc, ExitStack() as ctx:
        consts = ctx.enter_context(tc.tile_pool(name="consts", bufs=1))
        # PSUM: 'p2' 2x[128,1024]f32 (4 banks) + 'p1' 2x[128,512]f32
        # (2 banks) + 'ps' 2x 1-bank = exactly 8 banks.
        ps = ctx.enter_context(tc.tile_pool(name="ps", bufs=2, space="PSUM"))
        xs = ctx.enter_context(tc.tile_pool(name="xs", bufs=34))
        rp = ctx.enter_context(tc.tile_pool(name="rp", bufs=6))
        pts = ctx.enter_context(tc.tile_pool(name="pts", bufs=6))
        accp = ctx.enter_context(tc.tile_pool(name="accp", bufs=2))
        rcp = ctx.enter_context(tc.tile_pool(name="rcp", bufs=2))
        outs = ctx.enter_context(tc.tile_pool(name="outs", bufs=4))

        # ---- resident constants (issue order = need order; gpsimd SWDGE) ----
        wq_sb = consts.tile([128, NHT, DQ], BF16)
        wk_sb = consts.tile([128, NHT, HD], BF16)
        wv_sb = consts.tile([128, NHT, HD], BF16)
        wo_sb = consts.tile([128, HPG, H], BF16)
        pm_sb = consts.tile([128, HD], BF16)
        id_sb = consts.tile([128, 128], BF16)
        tm_sb = consts.tile([128, 128], BF16)
        on_sb = consts.tile([128, 1], BF16)
        onr_sb = consts.tile([1, 128], BF16)
        cos_sb = consts.tile([128, S], BF16)
        sin_sb = consts.tile([128, S], BF16)

        # resident activations (all bf16)
        kT_sb = consts.tile([128, S], BF16)
        v_sb = consts.tile([128, NST, HD], BF16)
        qT_sb = consts.tile([128, HPG, S], BF16)
        aT_sb = consts.tile([128, HPG, S], BF16)

        xts = {}  # (chunk, ht) -> xt tile

        def load_x_chunk(c):
            for ht in range(NHT):
                xt = xs.tile([128, SC], BF16, tag="xt", name=f"xt{c}_{ht}")
                xts[(c, ht)] = xt
                nc.sync.dma_start(out=xt, in_=xb[c, ht, :, :])

        # chunk 0 is startup-critical: even h-tiles on the sync queue, odd
        # h-tiles interleaved with wk/wv pieces on the gpsimd queue, in the
        # order pass 0 consumes them.
        for ht in range(NHT):
            xt = xs.tile([128, SC], BF16, tag="xt", name=f"xt0_{ht}")
            xts[(0, ht)] = xt
        for ht in range(0, NHT, 2):
            nc.sync.dma_start(out=xts[(0, ht)], in_=xb[0, ht, :, :])
        nc.gpsimd.dma_start(out=wk_sb[:, 0:4, :], in_=wk[:, 0:4, :])
        nc.gpsimd.dma_start(out=wv_sb[:, 0:4, :], in_=wv[:, 0:4, :])
        for ht in (1, 3):
            nc.gpsimd.dma_start(out=xts[(0, ht)], in_=xb[0, ht, :, :])
        nc.gpsimd.dma_start(out=wk_sb[:, 4:16, :], in_=wk[:, 4:16, :])
        nc.gpsimd.dma_start(out=wv_sb[:, 4:16, :], in_=wv[:, 4:16, :])
        for ht in (5, 7, 9, 11, 13, 15):
            nc.gpsimd.dma_start(out=xts[(0, ht)], in_=xb[0, ht, :, :])
        nc.gpsimd.dma_start(out=wq_sb[:, 0:4, :], in_=wq[:, 0:4, :])
        nc.gpsimd.dma_start(out=wq_sb[:, 4:10, :], in_=wq[:, 4:10, :])
        nc.gpsimd.dma_start(out=wq_sb[:, 10:16, :], in_=wq[:, 10:16, :])
        nc.gpsimd.dma_start(out=cos_sb, in_=cosT[:, :])
        nc.gpsimd.dma_start(out=sin_sb, in_=sinT[:, :])
        nc.gpsimd.dma_start(out=pm_sb, in_=pmat[:, :])
        nc.gpsimd.dma_start(out=id_sb, in_=ident[:, :])
        nc.gpsimd.dma_start(out=tm_sb, in_=trimask[:, :])
        nc.gpsimd.dma_start(out=on_sb, in_=ones[:, :])
        nc.gpsimd.dma_start(out=onr_sb, in_=onesrow[:, :])
        nc.gpsimd.dma_start(out=wo_sb[:, 0:2, :], in_=wo[:, 0:2, :])
        nc.gpsimd.dma_start(out=wo_sb[:, 2:4, :], in_=wo[:, 2:4, :])

        # ====== PE warmup ======
        # ~40 dummy matmuls keep the PE busy from engine boot (~5.5us) so
        # the HAM clock-gate reaches 2.4 GHz before the DMA-paced startup
        # ends (~18us); results go to a scratch PSUM slot and are unused.
        wup = rp.tile([128, SC], BF16, tag="wup", bufs=1, name="wup")
        nc.vector.memset(wup, 0.0)
        wdum = ps.tile([128, SC], F32, tag="p1", name="wdum")
        for _ in range(40):
            nc.tensor.matmul(wdum, wup[:, 0:128], wup, start=True, stop=True)

        # ====== projection chunk: K/V first, then Q pairs, RoPE ======
        def proj_chunk(c):
            s0 = c * SC
            if c + 1 < NSC:
                load_x_chunk(c + 1)

            def rope_tail(raw_slice, pqb_slice, dest):
                """qc(DVE) built from raw; u(gp) from pqb; dest = qc + u."""
                qc = rp.tile([128, SC], BF16, tag="qc", bufs=6, name="qc")
                nc.vector.tensor_mul(qc, raw_slice, cos_sb[:, s0:s0 + SC])
                u = rp.tile([128, SC], BF16, tag="u", bufs=3, name="u")
                nc.gpsimd.tensor_mul(u, pqb_slice, sin_sb[:, s0:s0 + SC])
                nc.gpsimd.tensor_add(dest, qc, u)

            # pass 0: K and V accumulators ('p1' ring)
            k_ps = ps.tile([128, SC], F32, tag="p1", name=f"kps{c}")
            v_ps = ps.tile([128, SC], F32, tag="p1", name=f"vps{c}")
            for ht in range(NHT):
                st = (ht == 0)
                sp = (ht == NHT - 1)
                xt = xts[(c, ht)]
                nc.tensor.matmul(k_ps, wk_sb[:, ht, :], xt, start=st, stop=sp)
                nc.tensor.matmul(v_ps, wv_sb[:, ht, :], xt, start=st, stop=sp)
            kraw = rp.tile([128, SC], BF16, tag="qraw1", bufs=2, name=f"kraw{c}")
            nc.scalar.copy(kraw, k_ps)          # ACT: PSUM f32 -> bf16
            vt = rp.tile([128, SC], BF16, tag="vt", bufs=2, name=f"vt{c}")
            nc.vector.tensor_copy(vt, v_ps)     # DVE cast for PE transposes

            # pass 1: Q heads 0/1 pair ('p2' ring); its evacuation overlaps
            # the pqk/transpose PE work below.
            qps = []
            qraws = []
            for pi in range(2):
                qps.append(ps.tile([128, 2 * SC], F32, tag="p2",
                                   name=f"qp{c}_{pi}"))

            def q_pass(pi):
                qp = qps[pi]
                for ht in range(NHT):
                    st = (ht == 0)
                    sp = (ht == NHT - 1)
                    xt = xts[(c, ht)]
                    for j in range(2):
                        hq = 2 * pi + j
                        nc.tensor.matmul(
                            qp[:, j * SC:(j + 1) * SC],
                            wq_sb[:, ht, hq * 128:(hq + 1) * 128], xt,
                            start=st, stop=sp)
                qraw = rp.tile([128, 2 * SC], BF16, tag="qraw2", bufs=2,
                               name=f"qraw{c}_{pi}")
                nc.scalar.copy(qraw, qp)        # one wide ACT evacuation
                qraws.append(qraw)

            q_pass(0)

            # K rope (kraw ready since pass 1 started) + V transposes:
            # PE work whose inputs are ready, placed between Q passes.
            pqk = ps.tile([128, SC], F32, tag="p1", name=f"pqk{c}")
            nc.tensor.matmul(pqk, pm_sb, kraw, start=True, stop=True)
            pqbk = rp.tile([128, SC], BF16, tag="pqb1", bufs=2, name=f"pqbk{c}")
            nc.scalar.copy(pqbk, pqk)
            for j in range(SC // 128):
                kt = c * (SC // 128) + j
                tr = ps.tile([128, 128], BF16, tag="ps", name=f"tr{c}_{j}")
                nc.tensor.transpose(tr, vt[:, j * 128:(j + 1) * 128], id_sb)
                nc.vector.tensor_copy(v_sb[:, kt, :], tr)
            rope_tail(kraw, pqbk, kT_sb[:, s0:s0 + SC])

            q_pass(1)

            # Q rope pairs
            for pi in range(2):
                qraw = qraws[pi]
                pq = ps.tile([128, 2 * SC], F32, tag="p2", name=f"pq{c}_{pi}")
                for j in range(2):
                    nc.tensor.matmul(
                        pq[:, j * SC:(j + 1) * SC], pm_sb,
                        qraw[:, j * SC:(j + 1) * SC], start=True, stop=True)
                pqb = rp.tile([128, 2 * SC], BF16, tag="pqb2", bufs=2,
                              name=f"pqb{c}_{pi}")
                nc.scalar.copy(pqb, pq)
                for j in range(2):
                    hq = 2 * pi + j
                    rope_tail(qraw[:, j * SC:(j + 1) * SC],
                              pqb[:, j * SC:(j + 1) * SC],
                              qT_sb[:, hq, s0:s0 + SC])

        # ====== attention head: paired scores^T -> one exp -> AV/sums ======
        fin_state = {}

        def attn_main(c, h):
            q0 = c * SC
            av = ps.tile([128, SC], F32, tag="p1", name=f"av{c}_{h}")
            sm = ps.tile([1, SC], F32, tag="ps", name=f"sm{c}_{h}")
            acc = [None]
            # pairs: [(kt, off, pos, W), ...] packed into one 2-bank psum.
            # FULL pairs first (their kT is from earlier chunks, ready
            # soonest, and they init the bf16 accumulator); diag pairs
            # last so their masked P^T can fold into the accumulator too.
            pairs = []
            d0 = 4 * c
            for i in range(2 * c):
                pairs.append([(2 * i, 0, 0, SC), (2 * i + 1, 0, SC, SC)])
            pairs.append([(d0 + 0, 0, 0, SC), (d0 + 1, 128, SC, SC - 128)])
            pairs.append([(d0 + 2, 256, 0, SC - 256), (d0 + 3, 384, SC - 256, SC - 384)])
            npairs = len(pairs)
            navs = 0
            pend = []
            for i, pair in enumerate(pairs):
                width = sum(p[3] for p in pair)
                diag = (i >= 2 * c)
                sps = ps.tile([128, 2 * SC], F32, tag="p2", name=f"sps{c}_{h}_{i}")
                for (kt, off, pos, W) in pair:
                    nc.tensor.matmul(
                        sps[:, pos:pos + W],
                        kT_sb[:, kt * 128:(kt + 1) * 128],
                        qT_sb[:, h, q0 + off:q0 + SC], start=True, stop=True)
                pt = pts.tile([128, 2 * SC], BF16, tag="pt", name=f"pt{c}_{h}_{i}")
                nc.scalar.activation(
                    out=pt[:, 0:width], in_=sps[:, 0:width],
                    func=mybir.ActivationFunctionType.Exp, scale=SCALE)
                if diag:
                    # causal: zero P^T where q < k in the first 128 q-cols.
                    # On GPSIMD (idle during attention) so the masked tiles
                    # don't queue behind DVE's accumulator adds and stall
                    # the diagonal AV matmuls.
                    for (kt, off, pos, W) in pair:
                        nc.gpsimd.tensor_mul(
                            pt[:, pos:pos + 128], pt[:, pos:pos + 128], tm_sb)
                    if c > 0:
                        # fold masked diag tiles into the accumulator on
                        # DVE (acc col j / 512+j both mean q=j) instead of
                        # PE ones-matmuls ([1,W] matmuls pay a ~115ns
                        # per-instruction penalty on top of W cycles).
                        for (kt, off, pos, W) in pair:
                            half = SC if pos else 0
                            nc.vector.tensor_add(
                                acc[0][:, half + off:half + off + W],
                                acc[0][:, half + off:half + off + W],
                                pt[:, pos:pos + W])
                else:
                    # full pairs: bf16 row-sum accumulation on DVE (2x mode)
                    if acc[0] is None:
                        acc[0] = accp.tile([128, 2 * SC], BF16, tag="accd",
                                           name=f"accd{c}_{h}")
                        nc.vector.tensor_copy(acc[0], pt)
                    else:
                        nc.vector.tensor_add(acc[0], acc[0], pt)

                def mk_post(pair=pair, pt=pt, diag=diag):
                    nonlocal navs
                    for (kt, off, pos, W) in pair:
                        nc.tensor.matmul(
                            av[:, off:SC], v_sb[:, kt, :], pt[:, pos:pos + W],
                            start=(navs == 0), stop=(navs == 2 * npairs - 1))
                        if diag and c == 0:
                            nc.tensor.matmul(
                                sm[:, off:SC], on_sb, pt[:, pos:pos + W],
                                start=(navs == 0), stop=(navs == 3))
                        navs += 1
                pend.append(mk_post)
                if len(pend) > 2:
                    pend.pop(0)()
            for f in pend:
                f()
            if acc[0] is not None:
                nc.tensor.matmul(sm, on_sb, acc[0][:, 0:SC],
                                 start=True, stop=False)
                nc.tensor.matmul(sm, on_sb, acc[0][:, SC:2 * SC],
                                 start=False, stop=True)
            # fin part A: 1/rowsum as exp(-ln(sum)) on ACT, issued right
            # away so rc16 is long ready when fin part B's PE matmul runs.
            # (Ln/Exp/Copy share the natural_log_exp_and_others ACT table.)
            lnr = rcp.tile([1, SC], F32, tag="lnr", name=f"lnr{c}_{h}")
            nc.scalar.activation(out=lnr, in_=sm,
                                 func=mybir.ActivationFunctionType.Ln)
            rc16 = rcp.tile([1, SC], BF16, tag="rc16", name=f"rc16{c}_{h}")
            nc.scalar.activation(out=rc16, in_=lnr,
                                 func=mybir.ActivationFunctionType.Exp,
                                 scale=-1.0)
            fin_state[(c, h)] = (av, rc16)

        def attn_fin(c, h):
            # fin part B (deferred past the next head's main): broadcast
            # the reciprocal and normalize.
            q0 = c * SC
            av, rc16 = fin_state.pop((c, h))
            rcb = ps.tile([128, SC], F32, tag="ps", name=f"rcb{c}_{h}")
            nc.tensor.matmul(rcb, onr_sb, rc16, start=True, stop=True)
            rcb_sb = rcp.tile([128, SC], BF16, tag="rcb", name=f"rcbs{c}_{h}")
            nc.vector.tensor_copy(rcb_sb, rcb)
            nc.vector.tensor_mul(aT_sb[:, h, q0:q0 + SC], av, rcb_sb)

        # ====== O-projection (paired output tiles, emitted in quarters) ======
        def oproj_part(c, nt_lo, nt_hi, alt=False):
            if c is None:
                return
            s0 = c * SC
            for nt in range(nt_lo, nt_hi, 2):
                yp = ps.tile([128, 2 * SC], F32, tag="p2", name=f"yp{c}_{nt}")
                for j in range(2):
                    for ct in range(HPG):
                        nc.tensor.matmul(
                            yp[:, j * SC:(j + 1) * SC],
                            wo_sb[:, ct, (nt + j) * 128:(nt + j + 1) * 128],
                            aT_sb[:, ct, s0:s0 + SC],
                            start=(ct == 0), stop=(ct == HPG - 1))
                yo = outs.tile([128, 2 * SC], BF16, tag="yo", name=f"yo{c}_{nt}")
                # in woven quarters, alternate the psum evacuation between
                # ACT and DVE so the 'p2' slot frees fast for the next
                # head's score pairs even when DVE is backed up.
                if alt and (nt // 2) % 2 == 0:
                    nc.scalar.copy(yo, yp)
                else:
                    nc.vector.tensor_copy(yo, yp)
                nc.sync.dma_start(out=yb[c, nt, :, :], in_=yo[:, 0:SC])
                nc.sync.dma_start(out=yb[c, nt + 1, :, :], in_=yo[:, SC:2 * SC])

        def attn_chunk_with_oproj(c, oc):
            """attn(c) with oproj(oc) quarters woven in. Each head's fin
            is deferred past the next head's main so the sum/reciprocal
            chain never stalls PE."""
            attn_main(c, 0)
            attn_main(c, 1)
            attn_fin(c, 0)
            oproj_part(oc, 0, 4, alt=True)
            attn_main(c, 2)
            attn_fin(c, 1)
            oproj_part(oc, 4, 8, alt=True)
            attn_main(c, 3)
            attn_fin(c, 2)
            oproj_part(oc, 8, 12, alt=True)
            attn_fin(c, 3)
            oproj_part(oc, 12, 16, alt=True)

        # ====== emission ======
        # PE executes its stream in order, so chain-light attention chunks
        # ascend (0..3) with oproj(c-1) woven in; o3 is a solid-PE tail.
        for c in range(NSC):
            proj_chunk(c)
        attn_chunk_with_oproj(0, None)
        attn_chunk_with_oproj(1, 0)
        attn_chunk_with_oproj(2, 1)
        attn_chunk_with_oproj(3, 2)
        oproj_part(3, 0, 16)

    _legalize_waits(nc)
    return nc


_NC = None


def _get_nc():
    global _NC
    if _NC is None:
        _NC = build_nc()
    return _NC


def _bf16(a):
    import ml_dtypes
    return np.ascontiguousarray(a, dtype=np.float32).astype(ml_dtypes.bfloat16)


def _host_tables():
    inv_freq = 1.0 / (THETA ** (np.arange(0, HD, 2, dtype=np.float32) / HD))
    t = np.arange(S, dtype=np.float32)
    freqs = np.outer(t, inv_freq)                       # [S, HD/2]
    emb = np.concatenate([freqs, freqs], axis=-1)       # [S, HD]
    cosT = np.ascontiguousarray(np.cos(emb).astype(np.float32).T)
    sinT = np.ascontiguousarray(np.sin(emb).astype(np.float32).T)
    pmat = np.zeros((HD, HD), dtype=np.float32)
    for dout in range(HD):
        if dout < HD // 2:
            pmat[dout + HD // 2, dout] = -1.0
        else:
            pmat[dout - HD // 2, dout] = 1.0
    ident = np.eye(128, dtype=np.float32)
    # trimask[kp, ql] = 1 where ql >= kp (upper triangular incl diagonal)
    tri = np.triu(np.ones((128, 128), dtype=np.float32))
    return cosT, sinT, pmat, ident, tri


def _make_in_maps(x, attention_mask, Wq, Wk, Wv, Wo):
    cosT, sinT, pmat, ident, tri = _host_tables()
    x = np.asarray(x, dtype=np.float32)
    Wq = np.asarray(Wq, dtype=np.float32)
    Wk = np.asarray(Wk, dtype=np.float32)
    Wv = np.asarray(Wv, dtype=np.float32)
    Wo = np.asarray(Wo, dtype=np.float32)

    in_maps = []
    for c in range(NCORES):
        b, g = divmod(c, GROUPS)
        xT = np.ascontiguousarray(x[b].T)                       # [H, S]
        xblk = xT.reshape(NHT, 128, NSC, SC).transpose(2, 0, 1, 3)
        wqT = Wq[g * DQ:(g + 1) * DQ, :].T                      # [H, DQ]
        wq_b = wqT.reshape(NHT, 128, DQ).transpose(1, 0, 2)
        wk_b = Wk[g * HD:(g + 1) * HD, :].T.reshape(NHT, 128, HD).transpose(1, 0, 2)
        wv_b = Wv[g * HD:(g + 1) * HD, :].T.reshape(NHT, 128, HD).transpose(1, 0, 2)
        woT = Wo[:, g * DQ:(g + 1) * DQ].T                      # [DQ, H]
        wo_b = woT.reshape(HPG, 128, H).transpose(1, 0, 2)
        in_maps.append({
            "xb": _bf16(xblk),
            "wq": _bf16(wq_b),
            "wk": _bf16(wk_b),
            "wv": _bf16(wv_b),
            "wo": _bf16(wo_b),
            "cosT": _bf16(cosT), "sinT": _bf16(sinT),
            "pmat": _bf16(pmat), "ident": _bf16(ident),
            "trimask": _bf16(tri),
            "ones": _bf16(np.ones((128, 1), dtype=np.float32)),
            "onesrow": _bf16(np.ones((1, 128), dtype=np.float32)),
        })
    return in_maps


def _mask_is_causal(attention_mask):
    m = np.asarray(attention_mask, dtype=np.float32)[0, 0]
    lower = np.tril(np.ones((S, S), dtype=bool))
    return bool(np.all(m[lower] == 0.0) and np.all(m[~lower] <= -1e8))


def _reference_fallback(x, attention_mask, Wq, Wk, Wv, Wo):
    """Numpy fallback for non-causal masks (never hit with the real harness)."""
    cosT, sinT, _, _, _ = _host_tables()
    cos, sin = cosT.T, sinT.T
    b, s, _ = x.shape
    q = (x @ Wq.T).reshape(b, s, NH, HD).transpose(0, 2, 1, 3)
    k = (x @ Wk.T).reshape(b, s, NKV, HD).transpose(0, 2, 1, 3)
    v = (x @ Wv.T).reshape(b, s, NKV, HD).transpose(0, 2, 1, 3)

    def rot(t):
        d = t.shape[-1] // 2
        return np.concatenate([-t[..., d:], t[..., :d]], axis=-1)

    q = q * cos + rot(q) * sin
    k = k * cos + rot(k) * sin
    k = np.repeat(k, NH // NKV, axis=1)
    v = np.repeat(v, NH // NKV, axis=1)
    sc = np.einsum("bhqd,bhkd->bhqk", q, k) / np.sqrt(np.float32(HD))
    sc = sc + np.asarray(attention_mask, dtype=np.float32)
    sc = sc - sc.max(axis=-1, keepdims=True)
    e = np.exp(sc)
    attn = e / e.sum(axis=-1, keepdims=True)
    out = np.einsum("bhqk,bhkd->bhqd", attn, v)
    out = out.transpose(0, 2, 1, 3).reshape(b, s, NH * HD)
    return (out @ Wo.T).astype(np.float32)


def _run(in_maps, trace=False, tmpdir=None):
    nc = _get_nc()
    kwargs = {}
    if trace:
        import trn_agent_boot.trn_boot as tb
        hook = tb._ntff_profile_via_ctypes("/opt/axon/libaxon_pjrt.so")
        m = types.ModuleType("antenv.axon_hooks")
        _h = {"hook": hook}
        m.get_axon_ntff_profile_hook = lambda: _h["hook"]
        m.set_axon_ntff_profile_hook = lambda h: _h.__setitem__("hook", h)
        sys.modules["antenv.axon_hooks"] = m
        kwargs = {"trace": True, "tmpdir": tmpdir}
    return run_bass_kernel_spmd(nc, in_maps, list(range(NCORES)), **kwargs)


def _assemble(results):
    out = np.empty((B, S, H), dtype=np.float32)
    for b in range(B):
        acc = results[b * GROUPS]["yb"].astype(np.float32)
        for g in range(1, GROUPS):
            acc = acc + results[b * GROUPS + g]["yb"].astype(np.float32)
        # acc: [NSC, NST, 128, SC] -> y^T [H, S] -> y [S, H]
        yT = acc.transpose(1, 2, 0, 3).reshape(H, S)
        out[b] = yT.T
    return out


def kernel(x, attention_mask, Wq, Wk, Wv, Wo):
    if not _mask_is_causal(attention_mask):
        return _reference_fallback(
            np.asarray(x, np.float32), attention_mask,
            np.asarray(Wq, np.float32), np.asarray(Wk, np.float32),
            np.asarray(Wv, np.float32), np.asarray(Wo, np.float32))
    in_maps = _make_in_maps(x, attention_mask, Wq, Wk, Wv, Wo)
    res = _run(in_maps)
    return _assemble(res.results)


# revision 30
# speedup vs baseline: 1.0795x; 1.0795x over previous
"""Self-contained Trainium2 Bass kernel for nn_CustomAttention_35278861369702.

Computation (see problem reference): causal GQA attention with RoPE.
  B=2, S=2048, H=2048, NH=16 q-heads, NKV=4 kv-heads, HD=128.

Sharding: 8 cores = 2 batches x 4 GQA groups. Core c handles batch c//4 and
q-heads 4g..4g+3 / kv-head g where g = c%4. Wq/Wk/Wv column-parallel,
Wo row-parallel; per-core partial outputs are summed on the host.

Device-side layout: everything transposed (no PE transposes of the softmax
matrix needed).
  - Projections produce Q^T/K^T/V^T [hd, s] (x^T as moving operand);
    chunk passes: (K,V) then (Q0,Q1) then (Q2,Q3) so attention data is
    ready earliest and startup DMA needs only wk/wv.
  - Scores computed as S^T [k, q] into PAIRED 2-bank PSUM tiles; ONE exp
    per pair on ACT ([128,1024] amortizes the 352-cycle ACT overhead);
    causal handled by trimming the moving q-range of diagonal k-tiles
    plus [128,128] triangular bf16 mask multiplies on DVE.
  - Softmax sums: diag tiles via cheap PE ones-matmuls; full-tile pairs
    accumulated in bf16 SBUF on GPSIMD (DVE stays free for evacuations);
    reciprocal on DVE (vector.reciprocal), broadcast via one PE matmul.
  - AV: lhsT=V tile [k, d], rhs=P^T [k, q] -> out^T [d, q], software
    pipelined so PE does not wait on ACT's exp.
  - O-projection paired (2 output tiles per 2-bank PSUM slot, one wide
    DVE cast), woven into the NEXT attention chunk; ascending chunk
    order (0..3) so every slot is PE-bound; oproj(3) is the tail.
  - All DRAM I/O uses host-prepacked contiguous blocks (full-rate DMA).
"""

import math
import sys
import types

sys.path.insert(0, "/opt/trn_rl_repo")

import numpy as np

import concourse.bass as bass
import concourse.mybir as mybir
import concourse.tile as tile
from concourse.bass_utils import run_bass_kernel_spmd

B, S, H = 2, 2048, 2048
NH, NKV, HD = 16, 4, 128
THETA = 10000.0
NCORES = 8
GROUPS = 4          # kv groups (= cores per batch)
HPG = NH // NKV     # q heads per group = 4
DQ = HPG * HD       # per-core q projection width = 512
SC = 512            # s-chunk (moving dim) for projections / attention
NSC = S // SC       # 4
NHT = H // 128      # 16 h-tiles (contraction tiles)
NST = S // 128      # 16 s-tiles / k-tiles
SCALE = 1.0 / math.sqrt(HD)

F32 = mybir.dt.float32
BF16 = mybir.dt.bfloat16


def _legalize_waits(nc):
    """This container's walrus accepts at most ONE sync wait per instruction.
    Split extra waits onto InstEventSemaphore carriers in engine order."""
    n = 0
    for f in nc.m.functions:
        for bb in f.blocks:
            new_insts = []
            for inst in bb.instructions:
                si = inst.sync_info
                if si and si.on_wait and len(si.on_wait) > 1:
                    waits = list(si.on_wait)
                    for j, w in enumerate(waits[:-1]):
                        es = mybir.InstEventSemaphore(
                            name=f"{inst.name}-wsplit{j}",
                            engine=inst.engine,
                            ins=[],
                            outs=[],
                            sync_info=mybir.SyncInfo(on_wait=[w], on_update=[]),
                        )
                        nc.register_instruction(es)
                        new_insts.append(es)
                        n += 1
                    si.on_wait = [waits[-1]]
                new_insts.append(inst)
            bb.instructions[:] = new_insts
    return n


def build_nc():
    nc = bass.Bass()

    # ---- DRAM I/O (per-core shards; same program on all 8 cores) ----
    # All blocks are host-prepacked to the exact SBUF layout so every DMA
    # is contiguous.
    xb = nc.dram_tensor("xb", [NSC, NHT, 128, SC], BF16, kind="ExternalInput")
    wq = nc.dram_tensor("wq", [128, NHT, DQ], BF16, kind="ExternalInput")
    wk = nc.dram_tensor("wk", [128, NHT, HD], BF16, kind="ExternalInput")
    wv = nc.dram_tensor("wv", [128, NHT, HD], BF16, kind="ExternalInput")
    wo = nc.dram_tensor("wo", [128, HPG, H], BF16, kind="ExternalInput")
    cosT = nc.dram_tensor("cosT", [128, S], BF16, kind="ExternalInput")
    sinT = nc.dram_tensor("sinT", [128, S], BF16, kind="ExternalInput")
    pmat = nc.dram_tensor("pmat", [HD, HD], BF16, kind="ExternalInput")
    ident = nc.dram_tensor("ident", [128, 128], BF16, kind="ExternalInput")
    trimask = nc.dram_tensor("trimask", [128, 128], BF16, kind="ExternalInput")
    ones = nc.dram_tensor("ones", [128, 1], BF16, kind="ExternalInput")
    onesrow = nc.dram_tensor("onesrow", [1, 128], BF16, kind="ExternalInput")

    yb = nc.dram_tensor("yb", [NSC, NST, 128, SC], BF16, kind="ExternalOutput")

    from contextlib import ExitStack

    with tile.TileContext(nc) as tc, ExitStack() as ctx:
        consts = ctx.enter_context(tc.tile_pool(name="consts", bufs=1))
        # PSUM: 'p2' 2x[128,1024]f32 (4 banks) + 'p1' 2x[128,512]f32
        # (2 banks) + 'ps' 2x 1-bank = exactly 8 banks.
        ps = ctx.enter_context(tc.tile_pool(name="ps", bufs=2, space="PSUM"))
        xs = ctx.enter_context(tc.tile_pool(name="xs", bufs=34))
        rp = ctx.enter_context(tc.tile_pool(name="rp", bufs=6))
        pts = ctx.enter_context(tc.tile_pool(name="pts", bufs=6))
        accp = ctx.enter_context(tc.tile_pool(name="accp", bufs=2))
        rcp = ctx.enter_context(tc.tile_pool(name="rcp", bufs=2))
        outs = ctx.enter_context(tc.tile_pool(name="outs", bufs=4))

        # ---- resident constants (issue order = need order; gpsimd SWDGE) ----
        wq_sb = consts.tile([128, NHT, DQ], BF16)
        wk_sb = consts.tile([128, NHT, HD], BF16)
        wv_sb = consts.tile([128, NHT, HD], BF16)
        wo_sb = consts.tile([128, HPG, H], BF16)
        pm_sb = consts.tile([128, HD], BF16)
        id_sb = consts.tile([128, 128], BF16)
        tm_sb = consts.tile([128, 128], BF16)
        on_sb = consts.tile([128, 1], BF16)
        onr_sb = consts.tile([1, 128], BF16)
        cos_sb = consts.tile([128, S], BF16)
        sin_sb = consts.tile([128, S], BF16)

        # resident activations (all bf16)
        kT_sb = consts.tile([128, S], BF16)
        v_sb = consts.tile([128, NST, HD], BF16)
        qT_sb = consts.tile([128, HPG, S], BF16)
        aT_sb = consts.tile([128, HPG, S], BF16)

        xts = {}  # (chunk, ht) -> xt tile

        def load_x_chunk(c):
            for ht in range(NHT):
                xt = xs.tile([128, SC], BF16, tag="xt", name=f"xt{c}_{ht}")
                xts[(c, ht)] = xt
                nc.sync.dma_start(out=xt, in_=xb[c, ht, :, :])

        # chunk 0 is startup-critical: even h-tiles on the sync queue, odd
        # h-tiles interleaved with wk/wv pieces on the gpsimd queue, in the
        # order pass 0 consumes them.
        for ht in range(NHT):
            xt = xs.tile([128, SC], BF16, tag="xt", name=f"xt0_{ht}")
            xts[(0, ht)] = xt
        for ht in range(0, NHT, 2):
            nc.sync.dma_start(out=xts[(0, ht)], in_=xb[0, ht, :, :])
        nc.gpsimd.dma_start(out=wk_sb[:, 0:4, :], in_=wk[:, 0:4, :])
        nc.gpsimd.dma_start(out=wv_sb[:, 0:4, :], in_=wv[:, 0:4, :])
        for ht in (1, 3):
            nc.gpsimd.dma_start(out=xts[(0, ht)], in_=xb[0, ht, :, :])
        nc.gpsimd.dma_start(out=wk_sb[:, 4:10, :], in_=wk[:, 4:10, :])
        nc.gpsimd.dma_start(out=wv_sb[:, 4:10, :], in_=wv[:, 4:10, :])
        for ht in (5, 7, 9):
            nc.gpsimd.dma_start(out=xts[(0, ht)], in_=xb[0, ht, :, :])
        nc.gpsimd.dma_start(out=wk_sb[:, 10:16, :], in_=wk[:, 10:16, :])
        nc.gpsimd.dma_start(out=wv_sb[:, 10:16, :], in_=wv[:, 10:16, :])
        for ht in (11, 13, 15):
            nc.gpsimd.dma_start(out=xts[(0, ht)], in_=xb[0, ht, :, :])
        nc.gpsimd.dma_start(out=wq_sb[:, 0:4, :], in_=wq[:, 0:4, :])
        nc.gpsimd.dma_start(out=wq_sb[:, 4:10, :], in_=wq[:, 4:10, :])
        nc.gpsimd.dma_start(out=wq_sb[:, 10:16, :], in_=wq[:, 10:16, :])
        nc.gpsimd.dma_start(out=cos_sb, in_=cosT[:, :])
        nc.gpsimd.dma_start(out=sin_sb, in_=sinT[:, :])
        nc.gpsimd.dma_start(out=pm_sb, in_=pmat[:, :])
        nc.gpsimd.dma_start(out=id_sb, in_=ident[:, :])
        nc.gpsimd.dma_start(out=tm_sb, in_=trimask[:, :])
        nc.gpsimd.dma_start(out=on_sb, in_=ones[:, :])
        nc.gpsimd.dma_start(out=onr_sb, in_=onesrow[:, :])
        nc.gpsimd.dma_start(out=wo_sb[:, 0:2, :], in_=wo[:, 0:2, :])
        nc.gpsimd.dma_start(out=wo_sb[:, 2:4, :], in_=wo[:, 2:4, :])

        # ====== PE warmup ======
        # ~40 dummy matmuls keep the PE busy from engine boot (~5.5us) so
        # the HAM clock-gate reaches 2.4 GHz before the DMA-paced startup
        # ends (~18us); results go to a scratch PSUM slot and are unused.
        wup = rp.tile([128, SC], BF16, tag="wup", bufs=1, name="wup")
        nc.vector.memset(wup, 0.0)
        wdum = ps.tile([128, SC], F32, tag="p1", name="wdum")
        for _ in range(40):
            nc.tensor.matmul(wdum, wup[:, 0:128], wup, start=True, stop=True)

        # ====== projection chunk: K/V first, then Q pairs, RoPE ======
        def proj_chunk(c):
            s0 = c * SC
            if c + 1 < NSC:
                load_x_chunk(c + 1)

            def rope_tail(raw_slice, pqb_slice, dest):
                """qc(DVE) built from raw; u(gp) from pqb; dest = qc + u."""
                qc = rp.tile([128, SC], BF16, tag="qc", bufs=6, name="qc")
                nc.vector.tensor_mul(qc, raw_slice, cos_sb[:, s0:s0 + SC])
                u = rp.tile([128, SC], BF16, tag="u", bufs=3, name="u")
                nc.gpsimd.tensor_mul(u, pqb_slice, sin_sb[:, s0:s0 + SC])
                nc.gpsimd.tensor_add(dest, qc, u)

            # pass 0: K and V accumulators ('p1' ring)
            k_ps = ps.tile([128, SC], F32, tag="p1", name=f"kps{c}")
            v_ps = ps.tile([128, SC], F32, tag="p1", name=f"vps{c}")
            for ht in range(NHT):
                st = (ht == 0)
                sp = (ht == NHT - 1)
                xt = xts[(c, ht)]
                nc.tensor.matmul(k_ps, wk_sb[:, ht, :], xt, start=st, stop=sp)
                nc.tensor.matmul(v_ps, wv_sb[:, ht, :], xt, start=st, stop=sp)
            kraw = rp.tile([128, SC], BF16, tag="qraw1", bufs=2, name=f"kraw{c}")
            nc.scalar.copy(kraw, k_ps)          # ACT: PSUM f32 -> bf16
            vt = rp.tile([128, SC], BF16, tag="vt", bufs=2, name=f"vt{c}")
            nc.vector.tensor_copy(vt, v_ps)     # DVE cast for PE transposes

            # pass 1: Q heads 0/1 pair ('p2' ring); its evacuation overlaps
            # the pqk/transpose PE work below.
            qps = []
            qraws = []
            for pi in range(2):
                qps.append(ps.tile([128, 2 * SC], F32, tag="p2",
                                   name=f"qp{c}_{pi}"))

            def q_pass(pi):
                qp = qps[pi]
                for ht in range(NHT):
                    st = (ht == 0)
                    sp = (ht == NHT - 1)
                    xt = xts[(c, ht)]
                    for j in range(2):
                        hq = 2 * pi + j
                        nc.tensor.matmul(
                            qp[:, j * SC:(j + 1) * SC],
                            wq_sb[:, ht, hq * 128:(hq + 1) * 128], xt,
                            start=st, stop=sp)
                qraw = rp.tile([128, 2 * SC], BF16, tag="qraw2", bufs=2,
                               name=f"qraw{c}_{pi}")
                nc.scalar.copy(qraw, qp)        # one wide ACT evacuation
                qraws.append(qraw)

            q_pass(0)

            # K rope (kraw ready since pass 1 started) + V transposes:
            # PE work whose inputs are ready, placed between Q passes.
            pqk = ps.tile([128, SC], F32, tag="p1", name=f"pqk{c}")
            nc.tensor.matmul(pqk, pm_sb, kraw, start=True, stop=True)
            pqbk = rp.tile([128, SC], BF16, tag="pqb1", bufs=2, name=f"pqbk{c}")
            nc.scalar.copy(pqbk, pqk)
            for j in range(SC // 128):
                kt = c * (SC // 128) + j
                tr = ps.tile([128, 128], BF16, tag="ps", name=f"tr{c}_{j}")
                nc.tensor.transpose(tr, vt[:, j * 128:(j + 1) * 128], id_sb)
                nc.vector.tensor_copy(v_sb[:, kt, :], tr)
            rope_tail(kraw, pqbk, kT_sb[:, s0:s0 + SC])

            q_pass(1)

            # Q rope pairs
            for pi in range(2):
                qraw = qraws[pi]
                pq = ps.tile([128, 2 * SC], F32, tag="p2", name=f"pq{c}_{pi}")
                for j in range(2):
                    nc.tensor.matmul(
                        pq[:, j * SC:(j + 1) * SC], pm_sb,
                        qraw[:, j * SC:(j + 1) * SC], start=True, stop=True)
                pqb = rp.tile([128, 2 * SC], BF16, tag="pqb2", bufs=2,
                              name=f"pqb{c}_{pi}")
                nc.scalar.copy(pqb, pq)
                for j in range(2):
                    hq = 2 * pi + j
                    rope_tail(qraw[:, j * SC:(j + 1) * SC],
                              pqb[:, j * SC:(j + 1) * SC],
                              qT_sb[:, hq, s0:s0 + SC])

        # ====== attention head: paired scores^T -> one exp -> AV/sums ======
        fin_state = {}

        def attn_main(c, h):
            q0 = c * SC
            av = ps.tile([128, SC], F32, tag="p1", name=f"av{c}_{h}")
            sm = ps.tile([1, SC], F32, tag="ps", name=f"sm{c}_{h}")
            acc = [None]
            # pairs: [(kt, off, pos, W), ...] packed into one 2-bank psum.
            # FULL pairs first (their kT is from earlier chunks, ready
            # soonest, and they init the bf16 accumulator); diag pairs
            # last so their masked P^T can fold into the accumulator too.
            pairs = []
            d0 = 4 * c
            for i in range(2 * c):
                pairs.append([(2 * i, 0, 0, SC), (2 * i + 1, 0, SC, SC)])
            pairs.append([(d0 + 0, 0, 0, SC), (d0 + 1, 128, SC, SC - 128)])
            pairs.append([(d0 + 2, 256, 0, SC - 256), (d0 + 3, 384, SC - 256, SC - 384)])
            npairs = len(pairs)
            navs = 0
            pend = []
            for i, pair in enumerate(pairs):
                width = sum(p[3] for p in pair)
                diag = (i >= 2 * c)
                sps = ps.tile([128, 2 * SC], F32, tag="p2", name=f"sps{c}_{h}_{i}")
                for (kt, off, pos, W) in pair:
                    nc.tensor.matmul(
                        sps[:, pos:pos + W],
                        kT_sb[:, kt * 128:(kt + 1) * 128],
                        qT_sb[:, h, q0 + off:q0 + SC], start=True, stop=True)
                pt = pts.tile([128, 2 * SC], BF16, tag="pt", name=f"pt{c}_{h}_{i}")
                nc.scalar.activation(
                    out=pt[:, 0:width], in_=sps[:, 0:width],
                    func=mybir.ActivationFunctionType.Exp, scale=SCALE)
                if diag:
                    # causal: zero P^T where q < k in the first 128 q-cols
                    for (kt, off, pos, W) in pair:
                        nc.vector.tensor_mul(
                            pt[:, pos:pos + 128], pt[:, pos:pos + 128], tm_sb)
                    if c > 0:
                        # fold masked diag tiles into the accumulator on
                        # DVE (acc col j / 512+j both mean q=j) instead of
                        # PE ones-matmuls ([1,W] matmuls pay a ~115ns
                        # per-instruction penalty on top of W cycles).
                        for (kt, off, pos, W) in pair:
                            half = SC if pos else 0
                            nc.vector.tensor_add(
                                acc[0][:, half + off:half + off + W],
                                acc[0][:, half + off:half + off + W],
                                pt[:, pos:pos + W])
                else:
                    # full pairs: bf16 row-sum accumulation on DVE (2x mode)
                    if acc[0] is None:
                        acc[0] = accp.tile([128, 2 * SC], BF16, tag="accd",
                                           name=f"accd{c}_{h}")
                        nc.vector.tensor_copy(acc[0], pt)
                    else:
                        nc.vector.tensor_add(acc[0], acc[0], pt)

                def mk_post(pair=pair, pt=pt, diag=diag):
                    nonlocal navs
                    for (kt, off, pos, W) in pair:
                        nc.tensor.matmul(
                            av[:, off:SC], v_sb[:, kt, :], pt[:, pos:pos + W],
                            start=(navs == 0), stop=(navs == 2 * npairs - 1))
                        if diag and c == 0:
                            nc.tensor.matmul(
                                sm[:, off:SC], on_sb, pt[:, pos:pos + W],
                                start=(navs == 0), stop=(navs == 3))
                        navs += 1
                pend.append(mk_post)
                if len(pend) > 2:
                    pend.pop(0)()
            for f in pend:
                f()
            if acc[0] is not None:
                nc.tensor.matmul(sm, on_sb, acc[0][:, 0:SC],
                                 start=True, stop=False)
                nc.tensor.matmul(sm, on_sb, acc[0][:, SC:2 * SC],
                                 start=False, stop=True)
            # fin part A: 1/rowsum as exp(-ln(sum)) on ACT, issued right
            # away so rc16 is long ready when fin part B's PE matmul runs.
            # (Ln/Exp/Copy share the natural_log_exp_and_others ACT table.)
            lnr = rcp.tile([1, SC], F32, tag="lnr", name=f"lnr{c}_{h}")
            nc.scalar.activation(out=lnr, in_=sm,
                                 func=mybir.ActivationFunctionType.Ln)
            rc16 = rcp.tile([1, SC], BF16, tag="rc16", name=f"rc16{c}_{h}")
            nc.scalar.activation(out=rc16, in_=lnr,
                                 func=mybir.ActivationFunctionType.Exp,
                                 scale=-1.0)
            fin_state[(c, h)] = (av, rc16)

        def attn_fin(c, h):
            # fin part B (deferred past the next head's main): broadcast
            # the reciprocal and normalize.
            q0 = c * SC
            av, rc16 = fin_state.pop((c, h))
            rcb = ps.tile([128, SC], F32, tag="ps", name=f"rcb{c}_{h}")
            nc.tensor.matmul(rcb, onr_sb, rc16, start=True, stop=True)
            rcb_sb = rcp.tile([128, SC], BF16, tag="rcb", name=f"rcbs{c}_{h}")
            nc.vector.tensor_copy(rcb_sb, rcb)
            nc.vector.tensor_mul(aT_sb[:, h, q0:q0 + SC], av, rcb_sb)

        # ====== O-projection (paired output tiles, emitted in quarters) ======
        def oproj_part(c, nt_lo, nt_hi, alt=False):
            if c is None:
                return
            s0 = c * SC
            for nt in range(nt_lo, nt_hi, 2):
                yp = ps.tile([128, 2 * SC], F32, tag="p2", name=f"yp{c}_{nt}")
                for j in range(2):
                    for ct in range(HPG):
                        nc.tensor.matmul(
                            yp[:, j * SC:(j + 1) * SC],
                            wo_sb[:, ct, (nt + j) * 128:(nt + j + 1) * 128],
                            aT_sb[:, ct, s0:s0 + SC],
                            start=(ct == 0), stop=(ct == HPG - 1))
                yo = outs.tile([128, 2 * SC], BF16, tag="yo", name=f"yo{c}_{nt}")
                # in woven quarters, alternate the psum evacuation between
                # ACT and DVE so the 'p2' slot frees fast for the next
                # head's score pairs even when DVE is backed up.
                if alt and (nt // 2) % 2 == 0:
                    nc.scalar.copy(yo, yp)
                else:
                    nc.vector.tensor_copy(yo, yp)
                nc.sync.dma_start(out=yb[c, nt, :, :], in_=yo[:, 0:SC])
                nc.sync.dma_start(out=yb[c, nt + 1, :, :], in_=yo[:, SC:2 * SC])

        def attn_chunk_with_oproj(c, oc):
            """attn(c) with oproj(oc) quarters woven in. Each head's fin
            is deferred past the next head's main so the sum/reciprocal
            chain never stalls PE."""
            attn_main(c, 0)
            attn_main(c, 1)
            attn_fin(c, 0)
            oproj_part(oc, 0, 4, alt=True)
            attn_main(c, 2)
            attn_fin(c, 1)
            oproj_part(oc, 4, 8, alt=True)
            attn_main(c, 3)
            attn_fin(c, 2)
            oproj_part(oc, 8, 12, alt=True)
            attn_fin(c, 3)
            oproj_part(oc, 12, 16, alt=True)

        # ====== emission ======
        # PE executes its stream in order, so chain-light attention chunks
        # ascend (0..3) with oproj(c-1) woven in; o3 is a solid-PE tail.
        for c in range(NSC):
            proj_chunk(c)
        attn_chunk_with_oproj(0, None)
        attn_chunk_with_oproj(1, 0)
        attn_chunk_with_oproj(2, 1)
        attn_chunk_with_oproj(3, 2)
        oproj_part(3, 0, 16)

    _legalize_waits(nc)
    return nc


_NC = None


def _get_nc():
    global _NC
    if _NC is None:
        _NC = build_nc()
    return _NC


def _bf16(a):
    import ml_dtypes
    return np.ascontiguousarray(a, dtype=np.float32).astype(ml_dtypes.bfloat16)


def _host_tables():
    inv_freq = 1.0 / (THETA ** (np.arange(0, HD, 2, dtype=np.float32) / HD))
    t = np.arange(S, dtype=np.float32)
    freqs = np.outer(t, inv_freq)                       # [S, HD/2]
    emb = np.concatenate([freqs, freqs], axis=-1)       # [S, HD]
    cosT = np.ascontiguousarray(np.cos(emb).astype(np.float32).T)
    sinT = np.ascontiguousarray(np.sin(emb).astype(np.float32).T)
    pmat = np.zeros((HD, HD), dtype=np.float32)
    for dout in range(HD):
        if dout < HD // 2:
            pmat[dout + HD // 2, dout] = -1.0
        else:
            pmat[dout - HD // 2, dout] = 1.0
    ident = np.eye(128, dtype=np.float32)
    # trimask[kp, ql] = 1 where ql >= kp (upper triangular incl diagonal)
    tri = np.triu(np.ones((128, 128), dtype=np.float32))
    return cosT, sinT, pmat, ident, tri


def _make_in_maps(x, attention_mask, Wq, Wk, Wv, Wo):
    cosT, sinT, pmat, ident, tri = _host_tables()
    x = np.asarray(x, dtype=np.float32)
    Wq = np.asarray(Wq, dtype=np.float32)
    Wk = np.asarray(Wk, dtype=np.float32)
    Wv = np.asarray(Wv, dtype=np.float32)
    Wo = np.asarray(Wo, dtype=np.float32)

    in_maps = []
    for c in range(NCORES):
        b, g = divmod(c, GROUPS)
        xT = np.ascontiguousarray(x[b].T)                       # [H, S]
        xblk = xT.reshape(NHT, 128, NSC, SC).transpose(2, 0, 1, 3)
        wqT = Wq[g * DQ:(g + 1) * DQ, :].T                      # [H, DQ]
        wq_b = wqT.reshape(NHT, 128, DQ).transpose(1, 0, 2)
        wk_b = Wk[g * HD:(g + 1) * HD, :].T.reshape(NHT, 128, HD).transpose(1, 0, 2)
        wv_b = Wv[g * HD:(g + 1) * HD, :].T.reshape(NHT, 128, HD).transpose(1, 0, 2)
        woT = Wo[:, g * DQ:(g + 1) * DQ].T                      # [DQ, H]
        wo_b = woT.reshape(HPG, 128, H).transpose(1, 0, 2)
        in_maps.append({
            "xb": _bf16(xblk),
            "wq": _bf16(wq_b),
            "wk": _bf16(wk_b),
            "wv": _bf16(wv_b),
            "wo": _bf16(wo_b),
            "cosT": _bf16(cosT), "sinT": _bf16(sinT),
            "pmat": _bf16(pmat), "ident": _bf16(ident),
            "trimask": _bf16(tri),
            "ones": _bf16(np.ones((128, 1), dtype=np.float32)),
            "onesrow": _bf16(np.ones((1, 128), dtype=np.float32)),
        })
    return in_maps


def _mask_is_causal(attention_mask):
    m = np.asarray(attention_mask, dtype=np.float32)[0, 0]
    lower = np.tril(np.ones((S, S), dtype=bool))
    return bool(np.all(m[lower] == 0.0) and np.all(m[~lower] <= -1e8))


def _reference_fallback(x, attention_mask, Wq, Wk, Wv, Wo):
    """Numpy fallback for non-causal masks (never hit with the real harness)."""
    cosT, sinT, _, _, _ = _host_tables()
    cos, sin = cosT.T, sinT.T
    b, s, _ = x.shape
    q = (x @ Wq.T).reshape(b, s, NH, HD).transpose(0, 2, 1, 3)
    k = (x @ Wk.T).reshape(b, s, NKV, HD).transpose(0, 2, 1, 3)
    v = (x @ Wv.T).reshape(b, s, NKV, HD).transpose(0, 2, 1, 3)

    def rot(t):
        d = t.shape[-1] // 2
        return np.concatenate([-t[..., d:], t[..., :d]], axis=-1)

    q = q * cos + rot(q) * sin
    k = k * cos + rot(k) * sin
    k = np.repeat(k, NH // NKV, axis=1)
    v = np.repeat(v, NH // NKV, axis=1)
    sc = np.einsum("bhqd,bhkd->bhqk", q, k) / np.sqrt(np.float32(HD))
    sc = sc + np.asarray(attention_mask, dtype=np.float32)
    sc = sc - sc.max(axis=-1, keepdims=True)
    e = np.exp(sc)
    attn = e / e.sum(axis=-1, keepdims=True)
    out = np.einsum("bhqk,bhkd->bhqd", attn, v)
    out = out.transpose(0, 2, 1, 3).reshape(b, s, NH * HD)
    return (out @ Wo.T).astype(np.float32)


def _run(in_maps, trace=False, tmpdir=None):
    nc = _get_nc()
    kwargs = {}
    if trace:
        import trn_agent_boot.trn_boot as tb
        hook = tb._ntff_profile_via_ctypes("/opt/axon/libaxon_pjrt.so")
        m = types.ModuleType("antenv.axon_hooks")
        _h = {"hook": hook}
        m.get_axon_ntff_profile_hook = lambda: _h["hook"]
        m.set_axon_ntff_profile_hook = lambda h: _h.__setitem__("hook", h)
        sys.modules["antenv.axon_hooks"] = m
        kwargs = {"trace": True, "tmpdir": tmpdir}
    return run_bass_kernel_spmd(nc, in_maps, list(range(NCORES)), **kwargs)


def _assemble(results):
    out = np.empty((B, S, H), dtype=np.float32)
    for b in range(B):
        acc = results[b * GROUPS]["yb"].astype(np.float32)
        for g in range(1, GROUPS):
            acc = acc + results[b * GROUPS + g]["yb"].astype(np.float32)
        # acc: [NSC, NST, 128, SC] -> y^T [H, S] -> y [S, H]
        yT = acc.transpose(1, 2, 0, 3).reshape(H, S)
        out[b] = yT.T
    return out


def kernel(x, attention_mask, Wq, Wk, Wv, Wo):
    if not _mask_is_causal(attention_mask):
        return _reference_fallback(
            np.asarray(x, np.float32), attention_mask,
            np.asarray(Wq, np.float32), np.asarray(Wk, np.float32),
            np.asarray(Wv, np.float32), np.asarray(Wo, np.float32))
    in_maps = _make_in_maps(x, attention_mask, Wq, Wk, Wv, Wo)
    res = _run(in_maps)
    return _assemble(res.results)


# revision 31
# speedup vs baseline: 1.0861x; 1.0061x over previous
"""Self-contained Trainium2 Bass kernel for nn_CustomAttention_35278861369702.

Computation (see problem reference): causal GQA attention with RoPE.
  B=2, S=2048, H=2048, NH=16 q-heads, NKV=4 kv-heads, HD=128.

Sharding: 8 cores = 2 batches x 4 GQA groups. Core c handles batch c//4 and
q-heads 4g..4g+3 / kv-head g where g = c%4. Wq/Wk/Wv column-parallel,
Wo row-parallel; per-core partial outputs are summed on the host.

Device-side layout: everything transposed (no PE transposes of the softmax
matrix needed).
  - Projections produce Q^T/K^T/V^T [hd, s] (x^T as moving operand);
    chunk passes: (K,V) then (Q0,Q1) then (Q2,Q3) so attention data is
    ready earliest and startup DMA needs only wk/wv.
  - Scores computed as S^T [k, q] into PAIRED 2-bank PSUM tiles; ONE exp
    per pair on ACT ([128,1024] amortizes the 352-cycle ACT overhead);
    causal handled by trimming the moving q-range of diagonal k-tiles
    plus [128,128] triangular bf16 mask multiplies on DVE.
  - Softmax sums: diag tiles via cheap PE ones-matmuls; full-tile pairs
    accumulated in bf16 SBUF on GPSIMD (DVE stays free for evacuations);
    reciprocal on DVE (vector.reciprocal), broadcast via one PE matmul.
  - AV: lhsT=V tile [k, d], rhs=P^T [k, q] -> out^T [d, q], software
    pipelined so PE does not wait on ACT's exp.
  - O-projection paired (2 output tiles per 2-bank PSUM slot, one wide
    DVE cast), woven into the NEXT attention chunk; ascending chunk
    order (0..3) so every slot is PE-bound; oproj(3) is the tail.
  - All DRAM I/O uses host-prepacked contiguous blocks (full-rate DMA).
"""

import math
import sys
import types

sys.path.insert(0, "/opt/trn_rl_repo")

import numpy as np

import concourse.bass as bass
import concourse.mybir as mybir
import concourse.tile as tile
from concourse.bass_utils import run_bass_kernel_spmd

B, S, H = 2, 2048, 2048
NH, NKV, HD = 16, 4, 128
THETA = 10000.0
NCORES = 8
GROUPS = 4          # kv groups (= cores per batch)
HPG = NH // NKV     # q heads per group = 4
DQ = HPG * HD       # per-core q projection width = 512
SC = 512            # s-chunk (moving dim) for projections / attention
NSC = S // SC       # 4
NHT = H // 128      # 16 h-tiles (contraction tiles)
NST = S // 128      # 16 s-tiles / k-tiles
SCALE = 1.0 / math.sqrt(HD)

F32 = mybir.dt.float32
BF16 = mybir.dt.bfloat16


def _legalize_waits(nc):
    """This container's walrus accepts at most ONE sync wait per instruction.
    Split extra waits onto InstEventSemaphore carriers in engine order."""
    n = 0
    for f in nc.m.functions:
        for bb in f.blocks:
            new_insts = []
            for inst in bb.instructions:
                si = inst.sync_info
                if si and si.on_wait and len(si.on_wait) > 1:
                    waits = list(si.on_wait)
                    for j, w in enumerate(waits[:-1]):
                        es = mybir.InstEventSemaphore(
                            name=f"{inst.name}-wsplit{j}",
                            engine=inst.engine,
                            ins=[],
                            outs=[],
                            sync_info=mybir.SyncInfo(on_wait=[w], on_update=[]),
                        )
                        nc.register_instruction(es)
                        new_insts.append(es)
                        n += 1
                    si.on_wait = [waits[-1]]
                new_insts.append(inst)
            bb.instructions[:] = new_insts
    return n


def build_nc():
    nc = bass.Bass()

    # ---- DRAM I/O (per-core shards; same program on all 8 cores) ----
    # All blocks are host-prepacked to the exact SBUF layout so every DMA
    # is contiguous.
    xb = nc.dram_tensor("xb", [NSC, NHT, 128, SC], BF16, kind="ExternalInput")
    wq = nc.dram_tensor("wq", [128, NHT, DQ], BF16, kind="ExternalInput")
    wk = nc.dram_tensor("wk", [128, NHT, HD], BF16, kind="ExternalInput")
    wv = nc.dram_tensor("wv", [128, NHT, HD], BF16, kind="ExternalInput")
    wo = nc.dram_tensor("wo", [128, HPG, H], BF16, kind="ExternalInput")
    cosT = nc.dram_tensor("cosT", [128, S], BF16, kind="ExternalInput")
    sinT = nc.dram_tensor("sinT", [128, S], BF16, kind="ExternalInput")
    pmat = nc.dram_tensor("pmat", [HD, HD], BF16, kind="ExternalInput")
    ident = nc.dram_tensor("ident", [128, 128], BF16, kind="ExternalInput")
    trimask = nc.dram_tensor("trimask", [128, 128], BF16, kind="ExternalInput")
    ones = nc.dram_tensor("ones", [128, 1], BF16, kind="ExternalInput")
    onesrow = nc.dram_tensor("onesrow", [1, 128], BF16, kind="ExternalInput")

    yb = nc.dram_tensor("yb", [NSC, NST, 128, SC], BF16, kind="ExternalOutput")

    from contextlib import ExitStack

    with tile.TileContext(nc) as tc, ExitStack() as ctx:
        consts = ctx.enter_context(tc.tile_pool(name="consts", bufs=1))
        # PSUM: 'p2' 2x[128,1024]f32 (4 banks) + 'p1' 2x[128,512]f32
        # (2 banks) + 'ps' 2x 1-bank = exactly 8 banks.
        ps = ctx.enter_context(tc.tile_pool(name="ps", bufs=2, space="PSUM"))
        xs = ctx.enter_context(tc.tile_pool(name="xs", bufs=34))
        rp = ctx.enter_context(tc.tile_pool(name="rp", bufs=6))
        pts = ctx.enter_context(tc.tile_pool(name="pts", bufs=6))
        accp = ctx.enter_context(tc.tile_pool(name="accp", bufs=2))
        rcp = ctx.enter_context(tc.tile_pool(name="rcp", bufs=2))
        outs = ctx.enter_context(tc.tile_pool(name="outs", bufs=4))

        # ---- resident constants (issue order = need order; gpsimd SWDGE) ----
        wq_sb = consts.tile([128, NHT, DQ], BF16)
        wk_sb = consts.tile([128, NHT, HD], BF16)
        wv_sb = consts.tile([128, NHT, HD], BF16)
        wo_sb = consts.tile([128, HPG, H], BF16)
        pm_sb = consts.tile([128, HD], BF16)
        id_sb = consts.tile([128, 128], BF16)
        tm_sb = consts.tile([128, 128], BF16)
        on_sb = consts.tile([128, 1], BF16)
        onr_sb = consts.tile([1, 128], BF16)
        cos_sb = consts.tile([128, S], BF16)
        sin_sb = consts.tile([128, S], BF16)

        # resident activations (all bf16)
        kT_sb = consts.tile([128, S], BF16)
        v_sb = consts.tile([128, NST, HD], BF16)
        qT_sb = consts.tile([128, HPG, S], BF16)
        aT_sb = consts.tile([128, HPG, S], BF16)

        xts = {}  # (chunk, ht) -> xt tile

        def load_x_chunk(c):
            for ht in range(NHT):
                xt = xs.tile([128, SC], BF16, tag="xt", name=f"xt{c}_{ht}")
                xts[(c, ht)] = xt
                nc.sync.dma_start(out=xt, in_=xb[c, ht, :, :])

        # chunk 0 is startup-critical: even h-tiles on the sync queue, odd
        # h-tiles interleaved with wk/wv pieces on the gpsimd queue, in the
        # order pass 0 consumes them.
        for ht in range(NHT):
            xt = xs.tile([128, SC], BF16, tag="xt", name=f"xt0_{ht}")
            xts[(0, ht)] = xt
        for ht in range(0, NHT, 2):
            nc.sync.dma_start(out=xts[(0, ht)], in_=xb[0, ht, :, :])
        nc.gpsimd.dma_start(out=wk_sb[:, 0:4, :], in_=wk[:, 0:4, :])
        nc.gpsimd.dma_start(out=wv_sb[:, 0:4, :], in_=wv[:, 0:4, :])
        for ht in (1, 3):
            nc.gpsimd.dma_start(out=xts[(0, ht)], in_=xb[0, ht, :, :])
        nc.gpsimd.dma_start(out=wk_sb[:, 4:10, :], in_=wk[:, 4:10, :])
        nc.gpsimd.dma_start(out=wv_sb[:, 4:10, :], in_=wv[:, 4:10, :])
        for ht in (5, 7, 9):
            nc.gpsimd.dma_start(out=xts[(0, ht)], in_=xb[0, ht, :, :])
        nc.gpsimd.dma_start(out=wk_sb[:, 10:16, :], in_=wk[:, 10:16, :])
        nc.gpsimd.dma_start(out=wv_sb[:, 10:16, :], in_=wv[:, 10:16, :])
        for ht in (11, 13, 15):
            nc.gpsimd.dma_start(out=xts[(0, ht)], in_=xb[0, ht, :, :])
        nc.gpsimd.dma_start(out=wq_sb[:, 0:4, :], in_=wq[:, 0:4, :])
        nc.gpsimd.dma_start(out=wq_sb[:, 4:10, :], in_=wq[:, 4:10, :])
        nc.gpsimd.dma_start(out=wq_sb[:, 10:16, :], in_=wq[:, 10:16, :])
        nc.gpsimd.dma_start(out=cos_sb, in_=cosT[:, :])
        nc.gpsimd.dma_start(out=sin_sb, in_=sinT[:, :])
        nc.gpsimd.dma_start(out=pm_sb, in_=pmat[:, :])
        nc.gpsimd.dma_start(out=id_sb, in_=ident[:, :])
        nc.gpsimd.dma_start(out=tm_sb, in_=trimask[:, :])
        nc.gpsimd.dma_start(out=on_sb, in_=ones[:, :])
        nc.gpsimd.dma_start(out=onr_sb, in_=onesrow[:, :])
        nc.gpsimd.dma_start(out=wo_sb[:, 0:2, :], in_=wo[:, 0:2, :])
        nc.gpsimd.dma_start(out=wo_sb[:, 2:4, :], in_=wo[:, 2:4, :])

        # ====== PE warmup ======
        # ~40 dummy matmuls keep the PE busy from engine boot (~5.5us) so
        # the HAM clock-gate reaches 2.4 GHz before the DMA-paced startup
        # ends (~18us); results go to a scratch PSUM slot and are unused.
        wup = rp.tile([128, SC], BF16, tag="wup", bufs=1, name="wup")
        nc.vector.memset(wup, 0.0)
        wdum = ps.tile([128, SC], F32, tag="p1", name="wdum")
        # 26 dummies end ~15.9us, just as the first real operands land
        # (~15.8us measured); 40 overshot and held the ready real matmul
        # back by ~3.9us.
        for _ in range(26):
            nc.tensor.matmul(wdum, wup[:, 0:128], wup, start=True, stop=True)

        # ====== projection chunk: K/V first, then Q pairs, RoPE ======
        def proj_chunk(c):
            s0 = c * SC
            if c + 1 < NSC:
                load_x_chunk(c + 1)

            def rope_tail(raw_slice, pqb_slice, dest):
                """qc(DVE) built from raw; u(gp) from pqb; dest = qc + u."""
                qc = rp.tile([128, SC], BF16, tag="qc", bufs=6, name="qc")
                nc.vector.tensor_mul(qc, raw_slice, cos_sb[:, s0:s0 + SC])
                u = rp.tile([128, SC], BF16, tag="u", bufs=3, name="u")
                nc.gpsimd.tensor_mul(u, pqb_slice, sin_sb[:, s0:s0 + SC])
                nc.gpsimd.tensor_add(dest, qc, u)

            # pass 0: K and V accumulators ('p1' ring)
            k_ps = ps.tile([128, SC], F32, tag="p1", name=f"kps{c}")
            v_ps = ps.tile([128, SC], F32, tag="p1", name=f"vps{c}")
            for ht in range(NHT):
                st = (ht == 0)
                sp = (ht == NHT - 1)
                xt = xts[(c, ht)]
                nc.tensor.matmul(k_ps, wk_sb[:, ht, :], xt, start=st, stop=sp)
                nc.tensor.matmul(v_ps, wv_sb[:, ht, :], xt, start=st, stop=sp)
            kraw = rp.tile([128, SC], BF16, tag="qraw1", bufs=2, name=f"kraw{c}")
            nc.scalar.copy(kraw, k_ps)          # ACT: PSUM f32 -> bf16
            vt = rp.tile([128, SC], BF16, tag="vt", bufs=2, name=f"vt{c}")
            nc.vector.tensor_copy(vt, v_ps)     # DVE cast for PE transposes

            # pass 1: Q heads 0/1 pair ('p2' ring); its evacuation overlaps
            # the pqk/transpose PE work below.
            qps = []
            qraws = []
            for pi in range(2):
                qps.append(ps.tile([128, 2 * SC], F32, tag="p2",
                                   name=f"qp{c}_{pi}"))

            def q_pass(pi):
                qp = qps[pi]
                for ht in range(NHT):
                    st = (ht == 0)
                    sp = (ht == NHT - 1)
                    xt = xts[(c, ht)]
                    for j in range(2):
                        hq = 2 * pi + j
                        nc.tensor.matmul(
                            qp[:, j * SC:(j + 1) * SC],
                            wq_sb[:, ht, hq * 128:(hq + 1) * 128], xt,
                            start=st, stop=sp)
                qraw = rp.tile([128, 2 * SC], BF16, tag="qraw2", bufs=2,
                               name=f"qraw{c}_{pi}")
                nc.scalar.copy(qraw, qp)        # one wide ACT evacuation
                qraws.append(qraw)

            q_pass(0)

            # K rope (kraw ready since pass 1 started) + V transposes:
            # PE work whose inputs are ready, placed between Q passes.
            pqk = ps.tile([128, SC], F32, tag="p1", name=f"pqk{c}")
            nc.tensor.matmul(pqk, pm_sb, kraw, start=True, stop=True)
            pqbk = rp.tile([128, SC], BF16, tag="pqb1", bufs=2, name=f"pqbk{c}")
            nc.scalar.copy(pqbk, pqk)
            for j in range(SC // 128):
                kt = c * (SC // 128) + j
                tr = ps.tile([128, 128], BF16, tag="ps", name=f"tr{c}_{j}")
                nc.tensor.transpose(tr, vt[:, j * 128:(j + 1) * 128], id_sb)
                nc.vector.tensor_copy(v_sb[:, kt, :], tr)
            rope_tail(kraw, pqbk, kT_sb[:, s0:s0 + SC])

            q_pass(1)

            # Q rope pairs
            for pi in range(2):
                qraw = qraws[pi]
                pq = ps.tile([128, 2 * SC], F32, tag="p2", name=f"pq{c}_{pi}")
                for j in range(2):
                    nc.tensor.matmul(
                        pq[:, j * SC:(j + 1) * SC], pm_sb,
                        qraw[:, j * SC:(j + 1) * SC], start=True, stop=True)
                pqb = rp.tile([128, 2 * SC], BF16, tag="pqb2", bufs=2,
                              name=f"pqb{c}_{pi}")
                nc.scalar.copy(pqb, pq)
                for j in range(2):
                    hq = 2 * pi + j
                    rope_tail(qraw[:, j * SC:(j + 1) * SC],
                              pqb[:, j * SC:(j + 1) * SC],
                              qT_sb[:, hq, s0:s0 + SC])

        # ====== attention head: paired scores^T -> one exp -> AV/sums ======
        fin_state = {}

        def attn_main(c, h):
            q0 = c * SC
            av = ps.tile([128, SC], F32, tag="p1", name=f"av{c}_{h}")
            sm = ps.tile([1, SC], F32, tag="ps", name=f"sm{c}_{h}")
            acc = [None]
            # pairs: [(kt, off, pos, W), ...] packed into one 2-bank psum.
            # FULL pairs first (their kT is from earlier chunks, ready
            # soonest, and they init the bf16 accumulator); diag pairs
            # last so their masked P^T can fold into the accumulator too.
            pairs = []
            d0 = 4 * c
            for i in range(2 * c):
                pairs.append([(2 * i, 0, 0, SC), (2 * i + 1, 0, SC, SC)])
            pairs.append([(d0 + 0, 0, 0, SC), (d0 + 1, 128, SC, SC - 128)])
            pairs.append([(d0 + 2, 256, 0, SC - 256), (d0 + 3, 384, SC - 256, SC - 384)])
            npairs = len(pairs)
            navs = 0
            pend = []
            for i, pair in enumerate(pairs):
                width = sum(p[3] for p in pair)
                diag = (i >= 2 * c)
                sps = ps.tile([128, 2 * SC], F32, tag="p2", name=f"sps{c}_{h}_{i}")
                for (kt, off, pos, W) in pair:
                    nc.tensor.matmul(
                        sps[:, pos:pos + W],
                        kT_sb[:, kt * 128:(kt + 1) * 128],
                        qT_sb[:, h, q0 + off:q0 + SC], start=True, stop=True)
                pt = pts.tile([128, 2 * SC], BF16, tag="pt", name=f"pt{c}_{h}_{i}")
                nc.scalar.activation(
                    out=pt[:, 0:width], in_=sps[:, 0:width],
                    func=mybir.ActivationFunctionType.Exp, scale=SCALE)
                if diag:
                    # causal: zero P^T where q < k in the first 128 q-cols
                    for (kt, off, pos, W) in pair:
                        nc.vector.tensor_mul(
                            pt[:, pos:pos + 128], pt[:, pos:pos + 128], tm_sb)
                    if c > 0:
                        # fold masked diag tiles into the accumulator on
                        # DVE (acc col j / 512+j both mean q=j) instead of
                        # PE ones-matmuls ([1,W] matmuls pay a ~115ns
                        # per-instruction penalty on top of W cycles).
                        for (kt, off, pos, W) in pair:
                            half = SC if pos else 0
                            nc.vector.tensor_add(
                                acc[0][:, half + off:half + off + W],
                                acc[0][:, half + off:half + off + W],
                                pt[:, pos:pos + W])
                else:
                    # full pairs: bf16 row-sum accumulation on DVE (2x mode)
                    if acc[0] is None:
                        acc[0] = accp.tile([128, 2 * SC], BF16, tag="accd",
                                           name=f"accd{c}_{h}")
                        nc.vector.tensor_copy(acc[0], pt)
                    else:
                        nc.vector.tensor_add(acc[0], acc[0], pt)

                def mk_post(pair=pair, pt=pt, diag=diag):
                    nonlocal navs
                    for (kt, off, pos, W) in pair:
                        nc.tensor.matmul(
                            av[:, off:SC], v_sb[:, kt, :], pt[:, pos:pos + W],
                            start=(navs == 0), stop=(navs == 2 * npairs - 1))
                        if diag and c == 0:
                            nc.tensor.matmul(
                                sm[:, off:SC], on_sb, pt[:, pos:pos + W],
                                start=(navs == 0), stop=(navs == 3))
                        navs += 1
                pend.append(mk_post)
                if len(pend) > 2:
                    pend.pop(0)()
            for f in pend:
                f()
            if acc[0] is not None:
                nc.tensor.matmul(sm, on_sb, acc[0][:, 0:SC],
                                 start=True, stop=False)
                nc.tensor.matmul(sm, on_sb, acc[0][:, SC:2 * SC],
                                 start=False, stop=True)
            # fin part A: 1/rowsum as exp(-ln(sum)) on ACT, issued right
            # away so rc16 is long ready when fin part B's PE matmul runs.
            # (Ln/Exp/Copy share the natural_log_exp_and_others ACT table.)
            lnr = rcp.tile([1, SC], F32, tag="lnr", name=f"lnr{c}_{h}")
            nc.scalar.activation(out=lnr, in_=sm,
                                 func=mybir.ActivationFunctionType.Ln)
            rc16 = rcp.tile([1, SC], BF16, tag="rc16", name=f"rc16{c}_{h}")
            nc.scalar.activation(out=rc16, in_=lnr,
                                 func=mybir.ActivationFunctionType.Exp,
                                 scale=-1.0)
            fin_state[(c, h)] = (av, rc16)

        def attn_fin(c, h):
            # fin part B (deferred past the next head's main): broadcast
            # the reciprocal and normalize.
            q0 = c * SC
            av, rc16 = fin_state.pop((c, h))
            rcb = ps.tile([128, SC], F32, tag="ps", name=f"rcb{c}_{h}")
            nc.tensor.matmul(rcb, onr_sb, rc16, start=True, stop=True)
            rcb_sb = rcp.tile([128, SC], BF16, tag="rcb", name=f"rcbs{c}_{h}")
            nc.vector.tensor_copy(rcb_sb, rcb)
            nc.vector.tensor_mul(aT_sb[:, h, q0:q0 + SC], av, rcb_sb)

        # ====== O-projection (paired output tiles, emitted in quarters) ======
        def oproj_part(c, nt_lo, nt_hi, alt=False):
            if c is None:
                return
            s0 = c * SC
            for nt in range(nt_lo, nt_hi, 2):
                yp = ps.tile([128, 2 * SC], F32, tag="p2", name=f"yp{c}_{nt}")
                for j in range(2):
                    for ct in range(HPG):
                        nc.tensor.matmul(
                            yp[:, j * SC:(j + 1) * SC],
                            wo_sb[:, ct, (nt + j) * 128:(nt + j + 1) * 128],
                            aT_sb[:, ct, s0:s0 + SC],
                            start=(ct == 0), stop=(ct == HPG - 1))
                yo = outs.tile([128, 2 * SC], BF16, tag="yo", name=f"yo{c}_{nt}")
                # in woven quarters, alternate the psum evacuation between
                # ACT and DVE so the 'p2' slot frees fast for the next
                # head's score pairs even when DVE is backed up.
                if alt and (nt // 2) % 2 == 0:
                    nc.scalar.copy(yo, yp)
                else:
                    nc.vector.tensor_copy(yo, yp)
                nc.sync.dma_start(out=yb[c, nt, :, :], in_=yo[:, 0:SC])
                nc.sync.dma_start(out=yb[c, nt + 1, :, :], in_=yo[:, SC:2 * SC])

        def attn_chunk_with_oproj(c, oc):
            """attn(c) with oproj(oc) quarters woven in. Each head's fin
            is deferred past the next head's main so the sum/reciprocal
            chain never stalls PE."""
            attn_main(c, 0)
            attn_main(c, 1)
            attn_fin(c, 0)
            oproj_part(oc, 0, 4, alt=True)
            attn_main(c, 2)
            attn_fin(c, 1)
            oproj_part(oc, 4, 8, alt=True)
            attn_main(c, 3)
            attn_fin(c, 2)
            oproj_part(oc, 8, 12, alt=True)
            attn_fin(c, 3)
            oproj_part(oc, 12, 16, alt=True)

        # ====== emission ======
        # PE executes its stream in order, so chain-light attention chunks
        # ascend (0..3) with oproj(c-1) woven in; o3 is a solid-PE tail.
        for c in range(NSC):
            proj_chunk(c)
        attn_chunk_with_oproj(0, None)
        attn_chunk_with_oproj(1, 0)
        attn_chunk_with_oproj(2, 1)
        attn_chunk_with_oproj(3, 2)
        oproj_part(3, 0, 16)

    _legalize_waits(nc)
    return nc


_NC = None


def _get_nc():
    global _NC
    if _NC is None:
        _NC = build_nc()
    return _NC


def _bf16(a):
    import ml_dtypes
    return np.ascontiguousarray(a, dtype=np.float32).astype(ml_dtypes.bfloat16)


def _host_tables():
    inv_freq = 1.0 / (THETA ** (np.arange(0, HD, 2, dtype=np.float32) / HD))
    t = np.arange(S, dtype=np.float32)
    freqs = np.outer(t, inv_freq)                       # [S, HD/2]
    emb = np.concatenate([freqs, freqs], axis=-1)       # [S, HD]
    cosT = np.ascontiguousarray(np.cos(emb).astype(np.float32).T)
    sinT = np.ascontiguousarray(np.sin(emb).astype(np.float32).T)
    pmat = np.zeros((HD, HD), dtype=np.float32)
    for dout in range(HD):
        if dout < HD // 2:
            pmat[dout + HD // 2, dout] = -1.0
        else:
            pmat[dout - HD // 2, dout] = 1.0
    ident = np.eye(128, dtype=np.float32)
    # trimask[kp, ql] = 1 where ql >= kp (upper triangular incl diagonal)
    tri = np.triu(np.ones((128, 128), dtype=np.float32))
    return cosT, sinT, pmat, ident, tri


def _make_in_maps(x, attention_mask, Wq, Wk, Wv, Wo):
    cosT, sinT, pmat, ident, tri = _host_tables()
    x = np.asarray(x, dtype=np.float32)
    Wq = np.asarray(Wq, dtype=np.float32)
    Wk = np.asarray(Wk, dtype=np.float32)
    Wv = np.asarray(Wv, dtype=np.float32)
    Wo = np.asarray(Wo, dtype=np.float32)

    in_maps = []
    for c in range(NCORES):
        b, g = divmod(c, GROUPS)
        xT = np.ascontiguousarray(x[b].T)                       # [H, S]
        xblk = xT.reshape(NHT, 128, NSC, SC).transpose(2, 0, 1, 3)
        wqT = Wq[g * DQ:(g + 1) * DQ, :].T                      # [H, DQ]
        wq_b = wqT.reshape(NHT, 128, DQ).transpose(1, 0, 2)
        wk_b = Wk[g * HD:(g + 1) * HD, :].T.reshape(NHT, 128, HD).transpose(1, 0, 2)
        wv_b = Wv[g * HD:(g + 1) * HD, :].T.reshape(NHT, 128, HD).transpose(1, 0, 2)
        woT = Wo[:, g * DQ:(g + 1) * DQ].T                      # [DQ, H]
        wo_b = woT.reshape(HPG, 128, H).transpose(1, 0, 2)
        in_maps.append({
            "xb": _bf16(xblk),
            "wq": _bf16(wq_b),
            "wk": _bf16(wk_b),
            "wv": _bf16(wv_b),
            "wo": _bf16(wo_b),
            "cosT": _bf16(cosT), "sinT": _bf16(sinT),
            "pmat": _bf16(pmat), "ident": _bf16(ident),
            "trimask": _bf16(tri),
            "ones": _bf16(np.ones((128, 1), dtype=np.float32)),
            "onesrow": _bf16(np.ones((1, 128), dtype=np.float32)),
        })
    return in_maps


def _mask_is_causal(attention_mask):
    m = np.asarray(attention_mask, dtype=np.float32)[0, 0]
    lower = np.tril(np.ones((S, S), dtype=bool))
    return bool(np.all(m[lower] == 0.0) and np.all(m[~lower] <= -1e8))


def _reference_fallback(x, attention_mask, Wq, Wk, Wv, Wo):
    """Numpy fallback for non-causal masks (never hit with the real harness)."""
    cosT, sinT, _, _, _ = _host_tables()
    cos, sin = cosT.T, sinT.T
    b, s, _ = x.shape
    q = (x @ Wq.T).reshape(b, s, NH, HD).transpose(0, 2, 1, 3)
    k = (x @ Wk.T).reshape(b, s, NKV, HD).transpose(0, 2, 1, 3)
    v = (x @ Wv.T).reshape(b, s, NKV, HD).transpose(0, 2, 1, 3)

    def rot(t):
        d = t.shape[-1] // 2
        return np.concatenate([-t[..., d:], t[..., :d]], axis=-1)

    q = q * cos + rot(q) * sin
    k = k * cos + rot(k) * sin
    k = np.repeat(k, NH // NKV, axis=1)
    v = np.repeat(v, NH // NKV, axis=1)
    sc = np.einsum("bhqd,bhkd->bhqk", q, k) / np.sqrt(np.float32(HD))
    sc = sc + np.asarray(attention_mask, dtype=np.float32)
    sc = sc - sc.max(axis=-1, keepdims=True)
    e = np.exp(sc)
    attn = e / e.sum(axis=-1, keepdims=True)
    out = np.einsum("bhqk,bhkd->bhqd", attn, v)
    out = out.transpose(0, 2, 1, 3).reshape(b, s, NH * HD)
    return (out @ Wo.T).astype(np.float32)


def _run(in_maps, trace=False, tmpdir=None):
    nc = _get_nc()
    kwargs = {}
    if trace:
        import trn_agent_boot.trn_boot as tb
        hook = tb._ntff_profile_via_ctypes("/opt/axon/libaxon_pjrt.so")
        m = types.ModuleType("antenv.axon_hooks")
        _h = {"hook": hook}
        m.get_axon_ntff_profile_hook = lambda: _h["hook"]
        m.set_axon_ntff_profile_hook = lambda h: _h.__setitem__("hook", h)
        sys.modules["antenv.axon_hooks"] = m
        kwargs = {"trace": True, "tmpdir": tmpdir}
    return run_bass_kernel_spmd(nc, in_maps, list(range(NCORES)), **kwargs)


def _assemble(results):
    out = np.empty((B, S, H), dtype=np.float32)
    for b in range(B):
        acc = results[b * GROUPS]["yb"].astype(np.float32)
        for g in range(1, GROUPS):
            acc = acc + results[b * GROUPS + g]["yb"].astype(np.float32)
        # acc: [NSC, NST, 128, SC] -> y^T [H, S] -> y [S, H]
        yT = acc.transpose(1, 2, 0, 3).reshape(H, S)
        out[b] = yT.T
    return out


def kernel(x, attention_mask, Wq, Wk, Wv, Wo):
    if not _mask_is_causal(attention_mask):
        return _reference_fallback(
            np.asarray(x, np.float32), attention_mask,
            np.asarray(Wq, np.float32), np.asarray(Wk, np.float32),
            np.asarray(Wv, np.float32), np.asarray(Wo, np.float32))
    in_maps = _make_in_maps(x, attention_mask, Wq, Wk, Wv, Wo)
    res = _run(in_maps)
    return _assemble(res.results)
